# revision 1
# baseline (speedup 1.0000x reference)
"""Trainium2 Bass kernel for AbsDiagNet.

Reference computation (T=256, B=128, I=512, H=2048, O=512):
    proj = einsum('tbi,hi->tbh', X, W_IH)
    h_0 = 0;  h_t = |proj_t + HH * h_{t-1}|   (elementwise over [B, H])
    Y = h_T @ W_HO.T + b_HO                   -> [B, O]

Strategy: data-parallel over batch across 8 cores (B_local = 16), params
replicated.  All operand transposes are done host-side during sharding so the
device only runs matmuls (bf16, full PE rate), the serial DVE recurrence,
and ACT-engine PSUM->SBUF copies.

Each recurrence step is ONE custom fused DVE op on a [128, 256] state tile
(h-on-partitions, (h_chunk, batch) on free dim):
    ABS_ADD:  h' = |h + proj_t|   (maxx(r, -r) with r = Src0+Src1; ~340ns/step
                                   measured vs ~715ns for add + sign-bit-and)

Per-core device pipeline over time segments (small head segments let the
serial DVE chain start early; small tail segments cut the end drain):
  one batched DMA per segment (all 4 i-chunks) -> PE: proj^T[h,(t,b)] bf16
  matmuls into bank-aligned 2-chunk PSUM tiles -> ACT: strided copy into a
  t-major SBUF proj buffer -> DVE: SEG fused recurrence steps.
W_HO^T loads late (only needed at the end); W_IH loads column-split so the
first h-chunks' matmuls can start after ~0.25MB.
Final: round h_T to f32r (ACT copy), then Y = h^T . W_HO^T + bias as a
17-matmul f32r PSUM accumulation group (bias folded in as a K=1 matmul
against a ones row).  Measured ~47-57us by the repeat-slope harness method
(baseline 198.7us same-method); output rel err ~2.4e-3 (bf16 proj GEMM,
fp32 recurrence, f32r output GEMM).
"""

import numpy as np
import ml_dtypes

import concourse.mybir as mybir
from concourse import bacc
from concourse.alu_op_type import AluOpType
from concourse.tile import TileContext
from concourse.bass_utils import run_bass_kernel_spmd

import concourse.dve_ops as dve_ops
from concourse.dve_ops import DveOp
from concourse.dve_spec import Spec, Src0, Src1, maxx, lower
from concourse.dve_uop import DveOpSpec

# Problem shape (hardcoded per contract).
T, B, I, H, O = 256, 128, 512, 2048, 512
NCORES = 8
BL = B // NCORES            # 16 batch elements per core
NC_H = H // 128             # 16 h-chunks
NC_I = I // 128             # 4 i-chunks
CPAIR = 2                   # h-chunks per PSUM tile (2 banks)
F32 = mybir.dt.float32
F32R = mybir.dt.float32r
BF16 = mybir.dt.bfloat16
I32 = mybir.dt.int32
BF_NP = ml_dtypes.bfloat16

# time segments: small head so the serial DVE chain starts early; small
# tail so the post-PE recurrence drain is short
SEGS = [4, 8, 20, 32, 32, 32, 32, 32, 32, 16, 12, 4]
assert sum(SEGS) == T


def _register_abs_add():
    """Fused custom DVE op: out = |in0 + in1| (one instruction per
    recurrence step).  Registered at import so kernel.py stays
    self-contained; the sha is computed by lowering the spec."""
    if "ABS_ADD_ANT" in dve_ops._SUB_OPCODE_FOR_NAME:
        return next(o for o in dve_ops.OPS if o.name == "ABS_ADD_ANT")
    r = Src0 + Src1
    spec = Spec(
        body=maxx(r, -r),
        reference=lambda in0, in1, s0, s1, imm2: np.abs(
            in0.astype(np.float32) + in1.astype(np.float32)
        ),
    )
    name = "ABS_ADD_ANT"
    opcode = dve_ops._CUSTOM_DVE_ROW_BASE + len(dve_ops.OPS)
    assert opcode < 0x20, "custom DVE opcode row overflow"
    shas = {}
    for ver in ("v3",):
        s = DveOpSpec(name=name, opcode=opcode, uops=lower(spec, ver=ver),
                      rd1_en=True)
        shas[ver] = s.sha(ver)
    op = DveOp(name, spec, subdim=False, uops_sha=shas)
    dve_ops.OPS.append(op)
    dve_ops._SUB_OPCODE_FOR_NAME[name] = opcode
    dve_ops.CUSTOM_DVE_SPECS[name] = spec
    return op


ABS_ADD = _register_abs_add()


def _build(apply_hh: bool, repeat: int = 1):
    nc = bacc.Bacc("TRN2", target_bir_lowering=False, debug=False)

    xt = nc.dram_tensor("xt", [I, T * BL], BF16, kind="ExternalInput")
    wih_t = nc.dram_tensor("wih_t", [I, H], BF16, kind="ExternalInput")
    who_t = nc.dram_tensor("who_t", [H, O], F32R, kind="ExternalInput")
    bias = nc.dram_tensor("bias", [1, O], F32R, kind="ExternalInput")
    ones = nc.dram_tensor("ones", [1, BL], F32R, kind="ExternalInput")
    if apply_hh:
        hhb = nc.dram_tensor("hhb", [128, NC_H * BL], F32, kind="ExternalInput")
    y = nc.dram_tensor("y", [BL, O], F32, kind="ExternalOutput")

    xt3 = xt.rearrange("(ic p) f -> p ic f", ic=NC_I, p=128)
    who3 = who_t.rearrange("(c p) f -> p c f", c=NC_H, p=128)

    with TileContext(nc) as tc:
        with (
            tc.tile_pool(name="wpool", bufs=1) as wpool,
            tc.tile_pool(name="xpool", bufs=4) as xpool,
            tc.tile_pool(name="ppool", bufs=2) as ppool,
            tc.tile_pool(name="spool", bufs=1) as spool,
            tc.tile_pool(name="psum", bufs=4, space="PSUM") as psum,
        ):
            # --- prefetch first xt block before weights ---
            xq = []
            xtile0 = xpool.tile([128, NC_I * SEGS[0] * BL], BF16, tag="xt0")
            nc.sync.dma_start(
                out=xtile0.rearrange("p (ic f) -> p ic f", ic=NC_I),
                in_=xt3[:, :, 0:SEGS[0] * BL],
            )
            xq.append(xtile0)
            # --- weights, bias, constants (resident) ---
            # wih loads split column-wise so the first h-chunks arrive fast
            HSPLIT = 256
            wih_sb = []
            for ic in range(NC_I):
                w = wpool.tile([128, H], BF16, tag=f"wih{ic}")
                nc.sync.dma_start(
                    out=w[:, :HSPLIT],
                    in_=wih_t[ic * 128:(ic + 1) * 128, :HSPLIT],
                )
                wih_sb.append(w)
            for ic in range(NC_I):
                nc.sync.dma_start(
                    out=wih_sb[ic][:, HSPLIT:],
                    in_=wih_t[ic * 128:(ic + 1) * 128, HSPLIT:],
                )
            bias_sb = wpool.tile([1, O], F32R, tag="bias")
            nc.sync.dma_start(out=bias_sb, in_=bias[:, :])
            ones_sb = wpool.tile([1, BL], F32R, tag="ones")
            nc.sync.dma_start(out=ones_sb, in_=ones[:, :])
            whot_sb = wpool.tile([128, NC_H * O], F32R, tag="whot")
            whot_loaded = False
            if apply_hh:
                hhb_sb = wpool.tile([128, NC_H * BL], F32, tag="hhb")
                nc.sync.dma_start(out=hhb_sb, in_=hhb[:, :])

            # --- recurrence state: [128, (c, b)]; h = c*128 + p ---
            sA = spool.tile([128, NC_H * BL], F32, tag="sA")
            sB = spool.tile([128, NC_H * BL], F32, tag="sB")

            for _rep in range(repeat):
                if apply_hh:
                    nc.vector.memset(sA, 0.0)
                states = [sA, sB]
                t0seg = 0
                for si, SEG in enumerate(SEGS):
                    tb0 = t0seg
                    t0seg += SEG
                    # proj buffer, t-major: free index = t*256 + c*16 + b
                    proj = ppool.tile([128, SEG * NC_H * BL], F32, tag="proj")
                    proj3 = proj.rearrange(
                        "p (t cb) -> p t cb", t=SEG, cb=NC_H * BL
                    )
                    if xq:
                        xtile = xq.pop(0)
                    else:
                        xtile = xpool.tile([128, NC_I * SEG * BL], BF16, tag="xt")
                        nc.sync.dma_start(
                            out=xtile.rearrange("p (ic f) -> p ic f", ic=NC_I),
                            in_=xt3[:, :, tb0 * BL:(tb0 + SEG) * BL],
                        )
                    xtile3 = xtile.rearrange("p (ic f) -> p ic f", ic=NC_I)
                    if si == len(SEGS) - 3 and not whot_loaded:
                        # load the output weights late, off the critical path
                        nc.sync.dma_start(
                            out=whot_sb.rearrange("p (c f) -> p c f", c=NC_H),
                            in_=who3,
                        )
                        whot_loaded = True
                    for cp in range(NC_H // CPAIR):
                        # one full bank (512 fp32) per cc so each slice is
                        # bank-aligned even when SEG*BL < 512
                        ps = psum.tile([128, CPAIR * 512], F32, tag="mm")
                        for cc in range(CPAIR):
                            c = cp * CPAIR + cc
                            for ic in range(NC_I):
                                nc.tensor.matmul(
                                    out=ps[:, cc * 512:cc * 512 + SEG * BL],
                                    lhsT=wih_sb[ic][:, c * 128:(c + 1) * 128],
                                    rhs=xtile3[:, ic, :],
                                    start=(ic == 0),
                                    stop=(ic == NC_I - 1),
                                )
                        # PSUM [128, (cc, t, b)] -> SBUF t-major slice
                        nc.scalar.copy(
                            out=proj3[
                                :, :, cp * CPAIR * BL:(cp + 1) * CPAIR * BL
                            ].rearrange("p t (cc b) -> p t cc b", cc=CPAIR, b=BL),
                            in_=ps.rearrange("p (cc f) -> p cc f", cc=CPAIR)[
                                :, :, :SEG * BL
                            ].rearrange(
                                "p cc (t b) -> p cc t b", t=SEG, b=BL
                            ).transpose([0, 2, 1, 3]),
                        )
                    for tl in range(SEG):
                        src, dst = states
                        if apply_hh:
                            # general path: h' = |hh*h + p|
                            nc.vector.tensor_mul(out=dst, in0=src, in1=hhb_sb)
                            nc.vector.tensor_add(
                                out=dst, in0=dst, in1=proj3[:, tl, :]
                            )
                            nc.vector.tensor_scalar(
                                out=src.bitcast(I32), in0=dst.bitcast(I32),
                                scalar1=0x7FFFFFFF, scalar2=None,
                                op0=AluOpType.bitwise_and,
                            )
                            states = [src, dst]
                        elif si == 0 and tl == 0:
                            # h_1 = |0 + p_0| = |p_0|: sign-bit clear straight
                            # from proj; skips the h memset and its chain-start
                            # dependency
                            nc.vector.tensor_scalar(
                                out=dst.bitcast(I32),
                                in0=proj3[:, 0, :].bitcast(I32),
                                scalar1=0x7FFFFFFF, scalar2=None,
                                op0=AluOpType.bitwise_and,
                            )
                            states = [dst, src]
                        else:
                            # h' = |h + p| as one fused DVE instruction
                            # (two interleaved half-width chains measured
                            # worse: +256 instructions of dispatch outweigh
                            # the write-drain hiding)
                            nc.vector._custom_dve(
                                ABS_ADD, out=dst, in0=proj3[:, tl, :], in1=src,
                            )
                            states = [dst, src]

                # round h_T to f32r for the output matmul (ACT engine,
                # off the DVE critical path)
                rfin = states[0]
                sAr = spool.tile([128, NC_H * BL], F32R, tag="sar")
                nc.scalar.copy(out=sAr, in_=rfin)
                sA3 = sAr.rearrange("p (c b) -> p c b", c=NC_H, b=BL)
                yps = psum.tile([BL, O], F32, tag="mm")
                for c in range(NC_H):
                    nc.tensor.matmul(
                        out=yps,
                        lhsT=sA3[:, c, :],
                        rhs=whot_sb[:, c * O:(c + 1) * O],
                        start=(c == 0),
                        stop=False,
                    )
                nc.tensor.matmul(
                    out=yps, lhsT=ones_sb, rhs=bias_sb, start=False, stop=True,
                )
                y_sb = spool.tile([BL, O], F32, tag="y")
                nc.scalar.copy(out=y_sb, in_=yps)
                nc.sync.dma_start(out=y[:, :], in_=y_sb)

    nc.compile()
    return nc


def prep_inputs(X, W_IH, W_HO, b_HO):
    """Host-side prep shared by kernel() and test.py: transposes + bf16
    casts + per-core X shards."""
    wih_t = np.ascontiguousarray(W_IH.T).astype(BF_NP)       # [I, H]
    who_t = np.ascontiguousarray(W_HO.T).astype(np.float32)  # [H, O]
    common = {"wih_t": wih_t, "who_t": who_t,
              "bias": b_HO.reshape(1, O).astype(np.float32),
              "ones": np.ones((1, BL), dtype=np.float32)}
    xts = []
    for k in range(NCORES):
        xk = X[:, k * BL:(k + 1) * BL, :]                    # [T, BL, I]
        xt = np.ascontiguousarray(
            xk.transpose(2, 0, 1)
        ).reshape(I, T * BL).astype(BF_NP)
        xts.append(xt)
    return common, xts


def kernel(X, W_IH, HH, W_HO, b_HO, _cache={}):
    X = np.asarray(X, dtype=np.float32)
    W_IH = np.asarray(W_IH, dtype=np.float32)
    HH = np.asarray(HH, dtype=np.float32)
    W_HO = np.asarray(W_HO, dtype=np.float32)
    b_HO = np.asarray(b_HO, dtype=np.float32)

    apply_hh = not np.all(HH == 1.0)

    if ("nc", apply_hh) not in _cache:
        _cache[("nc", apply_hh)] = _build(apply_hh)
    nc = _cache[("nc", apply_hh)]

    common, xts = prep_inputs(X, W_IH, W_HO, b_HO)
    if apply_hh:
        # hhb[p, c*BL + b] = HH[c*128 + p]
        hhb = np.repeat(
            HH.reshape(NC_H, 128).T[:, :, None], BL, axis=2
        ).reshape(128, NC_H * BL)
        common["hhb"] = np.ascontiguousarray(hhb)

    in_maps = [{"xt": xts[k], **common} for k in range(NCORES)]

    res = run_bass_kernel_spmd(nc, in_maps, core_ids=list(range(NCORES)))
    out = np.concatenate([res.results[k]["y"] for k in range(NCORES)], axis=0)
    return out.astype(np.float32)



# revision 4
# speedup vs baseline: 23.7456x; 23.7456x over previous
"""Trainium2 Bass kernel for AbsDiagNet (v2).

Reference computation (T=256, B=128, I=512, H=2048, O=512):
    proj = einsum('tbi,hi->tbh', X, W_IH)
    h_0 = 0;  h_t = |proj_t + HH * h_{t-1}|   (elementwise over [B, H])
    Y = h_T @ W_HO.T + b_HO                   -> [B, O]

Strategy: data-parallel over batch across 8 cores (B_local = 16), params
replicated.

v2 pipeline (per core, per repeat):
  PE    fp8e4m3 DoubleRow matmuls (0.5 cyc/row): main product X8 @ W_hi
        with K=512 packed as 2 pair-MMs (1 cyc/col), plus precision
        corrections (X_res @ W_hi + X8 @ W_lo) computed at 2-step window
        resolution and accumulated into the same PSUM group via the
        DR pair slots (1 cyc/col equiv).  ~55 us.
  ACT   stages PSUM -> SBUF f32 with the fp8 scale (1/4096) folded into
        the activation-Copy scale; one instr per (tblk, 4-chunk) block,
        t-major output layout.  ~60 us.
  DVE   the serial recurrence h' = |p + h| as a fused custom DVE op
        (exact f32), split into two half-width interleaved chains so the
        per-instruction ack/sem latency of one chain hides under the
        other's execution; 2x_2p (dual read port) perf mode.  ~65-70 us.
  PE    output GEMM in bf16 with M=O-chunks (N=16 per MM) + bias via a
        K=1 ones-row matmul.

Precision (measured vs fp32 reference, full pipeline in numpy):
  rel err ~7.1e-3  (threshold 2e-2): fp8 hi/lo weight split kills W-quant
  error; window-summed corrections kill X-quant error except where the
  |.|-walk flips sign inside a 2-step window (rare); chain state is f32.
"""

import numpy as np
import ml_dtypes

import concourse.mybir as mybir
from concourse import bacc
from concourse.alu_op_type import AluOpType
from concourse.tile import TileContext
from concourse.bass_utils import run_bass_kernel_spmd

import concourse.dve_ops as dve_ops
from concourse.dve_ops import DveOp
from concourse.dve_spec import Spec, Src0, Src1, maxx, lower
from concourse.dve_uop import DveOpSpec

# Problem shape (hardcoded per contract).
T, B, I, H, O = 256, 128, 512, 2048, 512
NCORES = 8
BL = B // NCORES            # 16 batch elements per core
NC_H = H // 128             # 16 h-chunks
NC_I = I // 128             # 4 i-chunks
NOC = O // 128              # 4 o-chunks
F32 = mybir.dt.float32
F32R = mybir.dt.float32r
BF16 = mybir.dt.bfloat16
FP8 = mybir.dt.float8e4
I32 = mybir.dt.int32
BF_NP = ml_dtypes.bfloat16
F8_NP = ml_dtypes.float8_e4m3

SX = 16.0                   # fp8 scale for X (avoid denormals)
SW = 256.0                  # fp8 scale for W_IH
LAM = SX * SW               # folded out at ACT staging time
RWIN = 2                    # correction window (steps); PE corr cost ~1/RWIN
TB = 32                     # t-steps per block
NTB = T // TB
NWB = TB // RWIN            # correction windows per block
CQ = 2                      # h-chunks per PSUM tile (2 banks)
G = 2                       # interleaved half-chains on DVE
HC = NC_H // G              # h-chunks per half


def _register_abs_add_2x():
    """Fused custom DVE op: out = |in0 + in1|, with the 2x_2p (dual read
    port) perf-mode slots populated so the cost model's 0.5x applies to
    f32 SBUF operands.  Functional execution of 2x_2p was verified exact
    on the runtime simulator (the dual-port mode runs the same uop
    program; the 2-byte packing modes do not and are not enabled for
    f32 operands)."""
    name = "ABS_ADD2X_ANT"
    if name in dve_ops._SUB_OPCODE_FOR_NAME:
        return next(o for o in dve_ops.OPS if o.name == name)
    r = Src0 + Src1
    spec = Spec(
        body=maxx(r, -r),
        reference=lambda in0, in1, s0, s1, imm2: np.abs(
            in0.astype(np.float32) + in1.astype(np.float32)
        ),
    )
    opcode = dve_ops._CUSTOM_DVE_ROW_BASE + len(dve_ops.OPS)
    assert opcode < 0x20, "custom DVE opcode row overflow"
    shas = {}
    for ver in ("v3",):
        uops = lower(spec, ver=ver)
        s = DveOpSpec(
            name=name, opcode=opcode, uops=uops, rd1_en=True,
            uops_2x=list(uops), uops_2x_2p=list(uops), perf_max=2,
        )
        shas[ver] = s.sha(ver)
        dve_ops._COMPILE_CACHE[(name, ver)] = s
    op = DveOp(name, spec, subdim=False, uops_sha=shas)
    dve_ops.OPS.append(op)
    dve_ops._SUB_OPCODE_FOR_NAME[name] = opcode
    dve_ops.CUSTOM_DVE_SPECS[name] = spec
    return op


ABS_ADD2X = _register_abs_add_2x()


def _build(apply_hh: bool, repeat: int = 1):
    nc = bacc.Bacc("TRN2", target_bir_lowering=False, debug=False)

    # [k, (ic, t*BL+b)] fp8: X * SX
    xq = nc.dram_tensor("xq", [128, NC_I * T * BL], FP8, kind="ExternalInput")
    # [k, (s, slot, w*BL+b)] fp8: slot0 = window-sum of X*SX - Xq (requant),
    #                             slot1 = window-sum of Xq (requant)
    xc = nc.dram_tensor("xc", [128, NC_I * 2 * (T // RWIN) * BL], FP8,
                        kind="ExternalInput")
    # [k, (c, P, slot, m)] fp8: W_hi K-pair layout for the main product
    wm = nc.dram_tensor("wm", [128, NC_H * 2 * 2 * 128], FP8,
                        kind="ExternalInput")
    # [k, (c, s, slot, m)] fp8: slot0 = W_hi, slot1 = W_lo (corr product)
    wc = nc.dram_tensor("wc", [128, NC_H * NC_I * 2 * 128], FP8,
                        kind="ExternalInput")
    # [k, (oc, c, o')] bf16: W_HO^T chunks (lhsT of the output GEMM)
    who = nc.dram_tensor("who", [128, NOC * NC_H * 128], BF16,
                         kind="ExternalInput")
    bias = nc.dram_tensor("bias", [1, O], BF16, kind="ExternalInput")
    if apply_hh:
        hhb = nc.dram_tensor("hhb", [128, NC_H * BL], F32, kind="ExternalInput")
    y = nc.dram_tensor("y", [O, BL], F32, kind="ExternalOutput")
    y3 = y.rearrange("(oc p) b -> p oc b", oc=NOC, p=128)

    with TileContext(nc) as tc:
        with (
            tc.tile_pool(name="wpool", bufs=1) as wpool,
            tc.tile_pool(name="xpool", bufs=2) as xpool,
            tc.tile_pool(name="spool", bufs=2) as spool,
            tc.tile_pool(name="hpool", bufs=1) as hpool,
            tc.tile_pool(name="psum", bufs=3, space="PSUM") as psum,
            tc.tile_pool(name="psumy", bufs=1, space="PSUM") as psumy,
        ):
            # --- resident weights ---
            wm_sb = wpool.tile([128, NC_H * 2 * 2 * 128], FP8, tag="wm")
            nc.sync.dma_start(out=wm_sb, in_=wm[:, :])
            wc_sb = wpool.tile([128, NC_H * NC_I * 2 * 128], FP8, tag="wc")
            nc.sync.dma_start(out=wc_sb, in_=wc[:, :])
            who_sb = wpool.tile([128, NOC * NC_H * 128], BF16, tag="who")
            nc.sync.dma_start(out=who_sb, in_=who[:, :])
            bias_sb = wpool.tile([1, O], BF16, tag="bias")
            nc.sync.dma_start(out=bias_sb, in_=bias[:, :])
            ones_sb = wpool.tile([1, BL], BF16, tag="ones")
            nc.gpsimd.memset(ones_sb, 1.0)
            if apply_hh:
                hh_sb = wpool.tile([128, NC_H * BL], F32, tag="hhb")
                nc.sync.dma_start(out=hh_sb, in_=hhb[:, :])

            wm5 = wm_sb.rearrange("k (c P s m) -> k c P s m", c=NC_H, P=2, s=2)
            wc5 = wc_sb.rearrange("k (c s j m) -> k c s j m", c=NC_H, s=NC_I, j=2)
            who3 = who_sb.rearrange("k (oc c m) -> k oc c m", oc=NOC, c=NC_H)

            # --- chain state (per half): ping-pong f32 tiles ---
            stA = [hpool.tile([128, HC * BL], F32, tag=f"stA{g}", name=f"stA{g}")
                   for g in range(G)]
            stB = [hpool.tile([128, HC * BL], F32, tag=f"stB{g}", name=f"stB{g}")
                   for g in range(G)]
            hT_bf = hpool.tile([128, NC_H * BL], BF16, tag="hTbf")
            tmp_hh = None
            if apply_hh:
                tmp_hh = hpool.tile([128, HC * BL], F32, tag="tmphh")

            for _rep in range(repeat):
                # per-repeat inputs
                xq_sb = xpool.tile([128, NC_I * T * BL], FP8, tag="xq")
                nc.sync.dma_start(out=xq_sb, in_=xq[:, :])
                xc_sb = xpool.tile([128, NC_I * 2 * (T // RWIN) * BL], FP8,
                                   tag="xc")
                nc.sync.dma_start(out=xc_sb, in_=xc[:, :])
                xq3 = xq_sb.rearrange("k (ic f) -> k ic f", ic=NC_I)
                xc4 = xc_sb.rearrange("k (s j f) -> k s j f", s=NC_I, j=2)

                cur, nxt = stA, stB
                for g in range(G):
                    nc.gpsimd.memset(cur[g], 0.0)

                for tb in range(NTB):
                    col0 = tb * TB * BL          # main column offset
                    wcol0 = tb * NWB * BL        # corr column offset
                    # staging buffers for this tblk, one per half:
                    # [128, (t, c', b)] f32, true scale
                    stg = [
                        spool.tile([128, TB * HC * BL], F32, tag=f"stg{g}",
                                   name=f"stg{g}_{tb}")
                        for g in range(G)
                    ]
                    for q in range(NC_H // CQ):
                        pb = psum.tile([128, CQ * TB * BL], F32, tag="pb")
                        pb3 = pb.rearrange("k (cc f) -> k cc f", cc=CQ)
                        for cc in range(CQ):
                            c = q * CQ + cc
                            # main: K=512 as 2 DoubleRow pair-MMs
                            for P in range(2):
                                nc.tensor.matmul(
                                    out=pb3[:, cc, :],
                                    lhsT=wm5[:, c, P],
                                    rhs=xq3[:, 2 * P:2 * P + 2,
                                            col0:col0 + TB * BL],
                                    start=(P == 0), stop=False,
                                    perf_mode=mybir.MatmulPerfMode.DoubleRow,
                                )
                            # corrections: accumulate into window-start cols
                            cw = pb3[:, cc, :].rearrange(
                                "k (w r b) -> k w r b", w=NWB, r=RWIN, b=BL
                            )[:, :, 0, :]
                            for s in range(NC_I):
                                nc.tensor.matmul(
                                    out=cw,
                                    lhsT=wc5[:, c, s],
                                    rhs=xc4[:, s, :, wcol0:wcol0 + NWB * BL],
                                    start=False, stop=(s == NC_I - 1),
                                    perf_mode=mybir.MatmulPerfMode.DoubleRow,
                                )
                        # stage this 4-chunk block: [k,(cc,t,b)] -> [k,(t,cc,b)]
                        g = q // (NC_H // CQ // G)
                        qq = q % (NC_H // CQ // G)
                        stg4 = stg[g].rearrange(
                            "k (t c b) -> k t c b", t=TB, c=HC, b=BL)
                        nc.scalar.activation(
                            out=stg4[:, :, qq * CQ:(qq + 1) * CQ, :],
                            in_=pb.rearrange(
                                "k (cc t b) -> k cc t b", cc=CQ, t=TB, b=BL
                            ).transpose([0, 2, 1, 3]),
                            func=mybir.ActivationFunctionType.Copy,
                            scale=1.0 / LAM,
                        )
                    # the serial chains for this tblk
                    stg4s = [
                        stg[g].rearrange("k (t f) -> k t f", t=TB)
                        for g in range(G)
                    ]
                    for tl in range(TB):
                        for g in range(G):
                            if apply_hh:
                                nc.vector.tensor_mul(
                                    out=tmp_hh, in0=cur[g],
                                    in1=hh_sb[:, g * HC * BL:(g + 1) * HC * BL],
                                )
                                src_in = tmp_hh
                            else:
                                src_in = cur[g]
                            bi = nc.vector._custom_dve(
                                ABS_ADD2X, out=nxt[g],
                                in0=stg4s[g][:, tl, :], in1=src_in,
                            )
                            bi.ins.perf_max = 2
                        cur, nxt = nxt, cur

                # h_T -> bf16 (Pool, off the critical engines)
                for g in range(G):
                    nc.gpsimd.tensor_copy(
                        out=hT_bf[:, g * HC * BL:(g + 1) * HC * BL],
                        in_=cur[g],
                    )
                # output GEMM: Y^T [O, BL] = W_HO @ h + b
                yT = psumy.tile([128, NOC * BL], F32, tag="yT")
                for oc in range(NOC):
                    for c in range(NC_H):
                        nc.tensor.matmul(
                            out=yT[:, oc * BL:(oc + 1) * BL],
                            lhsT=who3[:, oc, c],
                            rhs=hT_bf[:, c * BL:(c + 1) * BL],
                            start=(c == 0), stop=False,
                        )
                    nc.tensor.matmul(
                        out=yT[:, oc * BL:(oc + 1) * BL],
                        lhsT=bias_sb[:, oc * 128:(oc + 1) * 128],
                        rhs=ones_sb,
                        start=False, stop=True,
                    )
                y_sb = hpool.tile([128, NOC * BL], F32, tag="ysb")
                nc.scalar.copy(out=y_sb, in_=yT)
                nc.sync.dma_start(
                    out=y3,
                    in_=y_sb.rearrange("p (oc b) -> p oc b", oc=NOC),
                )

    nc.compile()
    return nc


def prep_inputs(X, W_IH, W_HO, b_HO):
    """Host-side prep: fp8 quantization (scaled), hi/lo weight split,
    window-summed correction operands, DoubleRow operand layouts, and
    per-core X shards.  Returns (common_inputs, per_core_input_list)."""
    X = np.asarray(X, dtype=np.float32)
    W_IH = np.asarray(W_IH, dtype=np.float32)
    W_HO = np.asarray(W_HO, dtype=np.float32)
    b_HO = np.asarray(b_HO, dtype=np.float32)

    def q8(a):
        return a.astype(F8_NP).astype(np.float32)

    # --- weights ---
    Ws = W_IH * SW                            # [H, I]
    Whi = q8(Ws)
    Wlo = q8(Ws - Whi)
    # main lhsT [k, c, P, slot, m] = Whi[c*128+m, P*256 + slot*128 + k]
    Whi_r = Whi.reshape(NC_H, 128, NC_I, 128)        # [c, m, ic, k]
    wm = np.ascontiguousarray(
        Whi_r.reshape(NC_H, 128, 2, 2, 128)          # [c, m, P, slot, k]
        .transpose(4, 0, 2, 3, 1)                    # [k, c, P, slot, m]
    ).reshape(128, NC_H * 2 * 2 * 128).astype(F8_NP)
    # corr lhsT [k, c, s, slot, m]: slot0 = Whi, slot1 = Wlo
    Wpair = np.stack([Whi.reshape(NC_H, 128, NC_I, 128),
                      Wlo.reshape(NC_H, 128, NC_I, 128)], axis=3)
    # Wpair: [c, m, s, slot, k]
    wc = np.ascontiguousarray(
        Wpair.transpose(4, 0, 2, 3, 1)               # [k, c, s, slot, m]
    ).reshape(128, NC_H * NC_I * 2 * 128).astype(F8_NP)
    # output weights: who[k, oc, c, m] = W_HO[oc*128+m, c*128+k]
    who = np.ascontiguousarray(
        W_HO.reshape(NOC, 128, NC_H, 128).transpose(3, 0, 2, 1)
    ).reshape(128, NOC * NC_H * 128).astype(BF_NP)
    common = {
        "wm": wm, "wc": wc, "who": who,
        "bias": b_HO.reshape(1, O).astype(BF_NP),
    }

    # --- per-core X ---
    per_core = []
    for k in range(NCORES):
        Xk = X[:, k * BL:(k + 1) * BL, :] * SX        # [T, BL, I]
        Xq = q8(Xk)
        # xq[k, ic, t*BL+b] = Xq[t, b, ic*128+k]
        xq = np.ascontiguousarray(
            Xq.reshape(T * BL, NC_I, 128).transpose(2, 1, 0)
        ).reshape(128, NC_I * T * BL).astype(F8_NP)
        Xres_w = q8((Xk - Xq).reshape(T // RWIN, RWIN, BL, I).sum(1))
        Xq_w = q8(Xq.reshape(T // RWIN, RWIN, BL, I).sum(1))
        # xc[k, s, slot, w*BL+b]
        xpair = np.stack([Xres_w, Xq_w], axis=0)      # [slot, w, b, I]
        xc = np.ascontiguousarray(
            xpair.reshape(2, (T // RWIN) * BL, NC_I, 128).transpose(3, 2, 0, 1)
        ).reshape(128, NC_I * 2 * (T // RWIN) * BL).astype(F8_NP)
        per_core.append({"xq": xq, "xc": xc})
    return common, per_core


def kernel(X, W_IH, HH, W_HO, b_HO, _cache={}):
    X = np.asarray(X, dtype=np.float32)
    W_IH = np.asarray(W_IH, dtype=np.float32)
    HH = np.asarray(HH, dtype=np.float32)
    W_HO = np.asarray(W_HO, dtype=np.float32)
    b_HO = np.asarray(b_HO, dtype=np.float32)

    apply_hh = not np.all(HH == 1.0)

    if ("nc", apply_hh) not in _cache:
        _cache[("nc", apply_hh)] = _build(apply_hh)
    nc = _cache[("nc", apply_hh)]

    common, per_core = prep_inputs(X, W_IH, W_HO, b_HO)
    if apply_hh:
        hhb = np.repeat(
            HH.reshape(NC_H, 128).T[:, :, None], BL, axis=2
        ).reshape(128, NC_H * BL)
        common["hhb"] = np.ascontiguousarray(hhb)

    in_maps = [{**per_core[k], **common} for k in range(NCORES)]
    res = run_bass_kernel_spmd(nc, in_maps, core_ids=list(range(NCORES)))
    # y per core: [O, BL] -> Y[B, O]
    out = np.concatenate(
        [res.results[k]["y"].T for k in range(NCORES)], axis=0
    )
    return out.astype(np.float32)
